# revision 46
# baseline (speedup 1.0000x reference)
"""Trainium2 Bass kernel for a 4-layer post-LN transformer encoder.

Sharding: 8 cores = 2 batch groups x 4-way sequence shard (256 tokens/core).
Per layer each core computes QKV for its own tokens, AllGathers K,V within
its 4-core batch group, then runs attention/FFN for its tokens only.
Activations are kept feature-major ([D, T]) on chip so every matmul consumes
natural layouts (weights as lhsT, activations as rhs) with zero transposes.
Matmuls run in bf16 (weights pre-cast on host), accumulation fp32 in PSUM;
softmax and layernorm run in fp32.

Attention per head: scores are computed transposed ([k_tokens, q_tokens]) so
the pad-mask (a per-key bias, i.e. per-partition) and the exp both fold into
the single PSUM-eviction activation. The softmax denominator falls out of the
ctx matmul via a ones-column interleaved into V; division uses a reciprocal
row broadcast across partitions with a tiny K=2 PE outer-product.
"""

import numpy as np
import ml_dtypes

import concourse.bass as bass
import concourse.mybir as mybir
import concourse.tile as tile
from concourse import bacc
from concourse.bass_utils import run_bass_kernel_spmd

# model dims (hardcoded per problem spec)
B, S, D, H, DK, DV, F, L, V = 2, 1024, 1024, 16, 64, 64, 4096, 4, 32000
PAD = 0
EPS = 1e-5
P = 128
NCORES = 8
GROUPS = [[0, 1, 2, 3], [4, 5, 6, 7]]
T = S // 4          # tokens per core
NC = D // P         # feature chunks (8)
FC = F // P         # ffn feature chunks (32)
NSH = 4             # shards per group
TC = T // P         # token chunks per core (2)
SCALE = 1.0 / np.sqrt(DK)

f32 = mybir.dt.float32
bf16 = mybir.dt.bfloat16

AF = mybir.ActivationFunctionType
ALU = mybir.AluOpType


def posenc_np(seq_len, dim):
    pos = np.arange(seq_len, dtype=np.float32)[:, None]
    div = np.exp(-(np.arange(0, dim, 2, dtype=np.float32) / dim) * np.log(10000.0))
    pe = np.zeros((seq_len, dim), np.float32)
    pe[:, 0::2] = np.sin(pos * div)
    pe[:, 1::2] = np.cos(pos * div)
    return pe


def build_nc():
    nc = bacc.Bacc(None, target_bir_lowering=False, num_devices=NCORES)

    # ---- DRAM parameters (per-core) ----
    x0t = nc.declare_dram_parameter("x0t", [NC, P, T], f32, isOutput=False)
    maskcol = nc.declare_dram_parameter("maskcol", [P, NC], f32, isOutput=False)
    wq = nc.declare_dram_parameter("wq", [L, D, D], bf16, isOutput=False)
    wk = nc.declare_dram_parameter("wk", [L, D, D], bf16, isOutput=False)
    wv = nc.declare_dram_parameter("wv", [L, D, D], bf16, isOutput=False)
    wo = nc.declare_dram_parameter("wo", [L, D, D], bf16, isOutput=False)
    w1 = nc.declare_dram_parameter("w1", [L, D, F], bf16, isOutput=False)
    w2 = nc.declare_dram_parameter("w2", [L, F, D], bf16, isOutput=False)
    out = nc.declare_dram_parameter("out", [NC, P, T], f32, isOutput=True)

    with tile.TileContext(nc) as tc:
        with (
            tc.tile_pool(name="persist", bufs=1) as persist,
            tc.tile_pool(name="wp", bufs=12) as wp,
            tc.tile_pool(name="w1p", bufs=10) as w1p,
            tc.tile_pool(name="w2p", bufs=6) as w2p,
            tc.tile_pool(name="pTp", bufs=3) as pTp,
            tc.tile_pool(name="rows", bufs=3) as rows,
            tc.tile_pool(name="ps_main", bufs=2, space="PSUM") as ps_main,
            tc.tile_pool(name="ps_attn", bufs=2, space="PSUM") as ps_attn,
            tc.tile_pool(name="ps_ctx", bufs=2, space="PSUM") as ps_ctx,
            tc.tile_pool(name="ps_aux", bufs=2, space="PSUM") as ps_aux,
            tc.tile_pool(name="dram", bufs=1, space="DRAM") as dram,
        ):
            # ---- persistent SBUF state ----
            x = persist.tile([P, NC, T], f32, name="x")           # residual stream
            xb = persist.tile([P, NC, T], bf16, name="xb")        # bf16 copy
            z = persist.tile([P, NC, T], f32, name="z")           # residual sum
            x1 = persist.tile([P, NC, T], f32, name="x1")         # post-LN1
            x1b = persist.tile([P, NC, T], bf16, name="x1b")
            zb = persist.tile([P, NC, T], bf16, name="zb")        # LN scratch
            sq = persist.tile([P, NC, T], bf16, name="sq")        # LN scratch
            qT = persist.tile([P, NC, T], bf16, name="qT")
            kTl = persist.tile([P, NC, T], bf16, name="kTl")      # local K^T
            vl = persist.tile([P, TC, D], bf16, name="vl")        # local V
            kT = persist.tile([P, NC, NSH, T], bf16, name="kT")   # gathered K^T
            vaug = persist.tile([P, NC, H * (DV + 1)], bf16, name="vaug")
            ctx_un = persist.tile([P, NC, T], bf16, name="ctx_un")
            ctxT = persist.tile([P, NC, T], bf16, name="ctxT")
            hT = persist.tile([P, FC, T], bf16, name="hT")
            mask_sb = persist.tile([P, NC], f32, name="mask_sb")
            ones_col = persist.tile([P, 1], bf16, name="ones_col")
            ones_row = persist.tile([1, P], f32, name="ones_row")
            ones_row_bf = persist.tile([1, P], bf16, name="ones_row_bf")
            eps1 = persist.tile([1, 1], f32, name="eps1")
            zcol = persist.tile([P, 1], f32, name="zcol")

            # DRAM bounce buffers for the per-group K and V AllGathers.
            # K is bounced p-major so both the bounce-in DMA and the gathered
            # load use full-row contiguous lines with no strided rearrange.
            k_in = dram.tile([P, NC, T], bf16, name="k_in")
            v_in = dram.tile([TC, P, D], bf16, name="v_in")
            k_out = dram.tile([NSH, P, NC, T], bf16, name="k_out")
            v_out = dram.tile([NSH, TC, P, D], bf16, name="v_out")

            # ---- prologue ----
            nc.sync.dma_start(x[:], x0t[:].rearrange("c p t -> p c t"))
            nc.sync.dma_start(mask_sb[:], maskcol[:])
            nc.vector.memset(ones_col[:], 1.0)
            nc.vector.memset(ones_row[:], 1.0)
            nc.vector.memset(ones_row_bf[:], 1.0)
            nc.vector.memset(eps1[:], EPS)
            nc.vector.memset(zcol[:], 0.0)
            # ones columns interleaved in vaug (written once; V DMAs skip them)
            nc.vector.memset(
                vaug[:].rearrange("p c (h e) -> p c h e", e=DV + 1)[:, :, :, DV:],
                1.0,
            )
            nc.vector.tensor_copy(xb[:], x[:])

            def ln_prep_chunk(m):
                """bf16 cast + square for chunk m (DVE/ACT, off the PE path)."""
                nc.scalar.copy(zb[:, m, :], z[:, m, :])
                nc.vector.tensor_mul(sq[:, m, :], zb[:, m, :], zb[:, m, :])

            def ln_stats_chunk(ps_mean, ps_sq, m):
                """Stats matmuls for chunk m (emitted lag-1 so PE never waits)."""
                nc.tensor.matmul(ps_mean[0:1, :T], lhsT=ones_col[:],
                                 rhs=zb[:, m, :], start=(m == 0), stop=(m == NC - 1))
                nc.tensor.matmul(ps_sq[0:1, :T], lhsT=ones_col[:],
                                 rhs=sq[:, m, :], start=(m == 0), stop=(m == NC - 1))

            def ln_tail(ps_mean, ps_sq, z_in, x_out, xb_out):
                """LayerNorm tail: row math + partition-broadcast + per-chunk
                normalize (per-chunk so the next GEMM phase starts early).
                rstd = exp(-0.5*ln(var+eps)) keeps ACT in the ln/exp table set."""
                m_row = rows.tile([1, T], f32, name="m_row")
                msq = rows.tile([1, T], f32, name="msq")
                var = rows.tile([1, T], f32, name="var")
                rstd = rows.tile([1, T], f32, name="rstd")
                mrs = rows.tile([1, T], f32, name="mrs")
                nc.vector.tensor_scalar_mul(m_row[:], ps_mean[0:1, :T], 1.0 / D)
                nc.vector.tensor_mul(msq[:], m_row[:], m_row[:])
                nc.vector.scalar_tensor_tensor(
                    var[:], in0=ps_sq[0:1, :T], scalar=1.0 / D, in1=msq[:],
                    op0=ALU.mult, op1=ALU.subtract)
                nc.scalar.activation(var[:], var[:], AF.Ln, bias=eps1[:], scale=1.0)
                nc.scalar.activation(rstd[:], var[:], AF.Exp, bias=zcol[0:1, :],
                                     scale=-0.5)
                nc.vector.tensor_mul(mrs[:], m_row[:], rstd[:])
                ps_r = ps_aux.tile([P, 512], f32, name="ps_r", tag="ax")
                ps_m2 = ps_aux.tile([P, 512], f32, name="ps_m2", tag="ax")
                nc.tensor.matmul(ps_r[:, :T], lhsT=ones_row[:], rhs=rstd[:],
                                 start=True, stop=True)
                nc.tensor.matmul(ps_m2[:, :T], lhsT=ones_row[:], rhs=mrs[:],
                                 start=True, stop=True)
                for c in range(NC):
                    nc.vector.tensor_mul(x_out[:, c, :], z_in[:, c, :], ps_r[:, :T])
                    nc.vector.tensor_sub(x_out[:, c, :], x_out[:, c, :], ps_m2[:, :T])
                    nc.scalar.copy(xb_out[:, c, :], x_out[:, c, :])

            # ---- layers ----
            for l in range(L):
                # --- K, V projections first (they feed the collective) ---
                wk_sb = [wp.tile([P, D], bf16, name=f"wk_{l}_{c}", tag="w")
                         for c in range(NC)]
                for c in range(NC):
                    nc.sync.dma_start(wk_sb[c][:], wk[l, c * P:(c + 1) * P, :])
                for m in range(NC):
                    ps = ps_main.tile([P, 512], f32, name="ps_k", tag="mm")
                    for c in range(NC):
                        nc.tensor.matmul(ps[:, :T], lhsT=wk_sb[c][:, m * P:(m + 1) * P],
                                         rhs=xb[:, c, :], start=(c == 0), stop=(c == NC - 1))
                    nc.scalar.copy(kTl[:, m, :], ps[:, :T])
                nc.sync.dma_start(k_in[:], kTl[:])
                nc.gpsimd.collective_compute(
                    "AllGather", ALU.bypass, replica_groups=GROUPS,
                    ins=[k_in.opt()], outs=[k_out.opt()])

                wv_sb = [wp.tile([P, D], bf16, name=f"wv_{l}_{c}", tag="w")
                         for c in range(NC)]
                for c in range(NC):
                    nc.sync.dma_start(wv_sb[c][:], wv[l, c * P:(c + 1) * P, :])
                for t in range(TC):
                    for nh in range(2):
                        ps = ps_main.tile([P, 512], f32, name="ps_v", tag="mm")
                        for c in range(NC):
                            nc.tensor.matmul(
                                ps[:], lhsT=xb[:, c, t * P:(t + 1) * P],
                                rhs=wv_sb[c][:, nh * 512:(nh + 1) * 512],
                                start=(c == 0), stop=(c == NC - 1))
                        nc.scalar.copy(vl[:, t, nh * 512:(nh + 1) * 512], ps[:])
                    nc.sync.dma_start(v_in[t], vl[:, t, :])
                nc.gpsimd.collective_compute(
                    "AllGather", ALU.bypass, replica_groups=GROUPS,
                    ins=[v_in.opt()], outs=[v_out.opt()])

                # --- Q projection (overlaps with the AllGather) ---
                wq_sb = [wp.tile([P, D], bf16, name=f"wq_{l}_{c}", tag="w")
                         for c in range(NC)]
                for c in range(NC):
                    nc.sync.dma_start(wq_sb[c][:], wq[l, c * P:(c + 1) * P, :])
                for m in range(NC):
                    ps = ps_main.tile([P, 512], f32, name="ps_q", tag="mm")
                    for c in range(NC):
                        nc.tensor.matmul(ps[:, :T], lhsT=wq_sb[c][:, m * P:(m + 1) * P],
                                         rhs=xb[:, c, :], start=(c == 0), stop=(c == NC - 1))
                    nc.scalar.copy(qT[:, m, :], ps[:, :T])

                # --- pull this group's gathered K/V shards into SBUF ---
                for sh in range(NSH):
                    nc.sync.dma_start(kT[:, :, sh, :], k_out[sh])
                for kc in range(NC):
                    sh, j = kc // 2, kc % 2
                    nc.sync.dma_start(
                        vaug[:, kc, :].rearrange("p (h e) -> p h e", e=DV + 1)[:, :, :DV],
                        v_out[sh, j].rearrange("p (h e) -> p h e", e=DV))

                # --- attention ---
                e = DV + 1
                ps_b = None
                for h in range(H):
                    po = (h % 2) * DV
                    cc = h // 2
                    pT = pTp.tile([P, NC, T], bf16, name="pT")
                    for kc in range(NC):
                        sh, j = kc // 2, kc % 2
                        ps_s = ps_attn.tile([P, T], f32, name="ps_s", tag="sc")
                        nc.tensor.matmul(
                            ps_s[:],
                            lhsT=kT[po:po + DV, cc, sh, j * P:(j + 1) * P],
                            rhs=qT[po:po + DV, cc, :], start=True, stop=True)
                        nc.scalar.activation(pT[:, kc, :], ps_s[:], AF.Exp,
                                             bias=mask_sb[:, kc:kc + 1], scale=1.0)
                    ps_c = ps_ctx.tile([P, T], f32, name="ps_c", tag="cx")
                    for kc in range(NC):
                        nc.tensor.matmul(
                            ps_c[:e, :], lhsT=vaug[:, kc, h * e:(h + 1) * e],
                            rhs=pT[:, kc, :], start=(kc == 0), stop=(kc == NC - 1))
                    rp = rows.tile([1, T], f32, name="rp")
                    rp_bf = rows.tile([1, T], bf16, name="rp_bf")
                    nc.vector.reciprocal(rp[:], ps_c[DV:e, :])
                    nc.vector.tensor_copy(rp_bf[:], rp[:])
                    nc.vector.tensor_copy(ctx_un[po:po + DV, cc, :], ps_c[:DV, :])
                    if h % 2 == 0:
                        ps_b = ps_aux.tile([P, 512], f32, name="ps_b", tag="ax")
                    nc.tensor.matmul(ps_b[po:po + DV, :T],
                                     lhsT=ones_row_bf[0:1, :DV], rhs=rp_bf[:],
                                     start=True, stop=True)
                    if h % 2 == 1:
                        i = h // 2
                        nc.vector.tensor_mul(ctxT[:, i, :], ctx_un[:, i, :],
                                             ps_b[:, :T])

                # --- Wo + residual + LN1 ---
                wo_sb = [wp.tile([P, D], bf16, name=f"wo_{l}_{c}", tag="w")
                         for c in range(NC)]
                for c in range(NC):
                    nc.sync.dma_start(wo_sb[c][:], wo[l, c * P:(c + 1) * P, :])
                ps_mean = ps_aux.tile([P, 512], f32, name="ps_mean", tag="ax")
                ps_sq = ps_aux.tile([P, 512], f32, name="ps_sq", tag="ax")
                for m in range(NC):
                    ps = ps_main.tile([P, 512], f32, name="ps_o", tag="mm")
                    for c in range(NC):
                        nc.tensor.matmul(ps[:, :T], lhsT=wo_sb[c][:, m * P:(m + 1) * P],
                                         rhs=ctxT[:, c, :], start=(c == 0), stop=(c == NC - 1))
                    nc.vector.tensor_add(z[:, m, :], ps[:, :T], x[:, m, :])
                    ln_prep_chunk(m)
                    if m >= 1:
                        ln_stats_chunk(ps_mean, ps_sq, m - 1)
                ln_stats_chunk(ps_mean, ps_sq, NC - 1)
                ln_tail(ps_mean, ps_sq, z, x1, x1b)

                # --- FFN1 (+gelu) ---
                for g in range(4):
                    w1_sb = [w1p.tile([P, D], bf16, name=f"w1_{l}_{g}_{c}", tag="w1")
                             for c in range(NC)]
                    for c in range(NC):
                        nc.sync.dma_start(
                            w1_sb[c][:], w1[l, c * P:(c + 1) * P, g * D:(g + 1) * D])
                    for mf_l in range(0, NC, 2):
                        mf = g * NC + mf_l
                        ps = ps_main.tile([P, 512], f32, name="ps_f1", tag="mm")
                        for half in range(2):
                            for c in range(NC):
                                nc.tensor.matmul(
                                    ps[:, half * T:(half + 1) * T],
                                    lhsT=w1_sb[c][:, (mf_l + half) * P:
                                                  (mf_l + half + 1) * P],
                                    rhs=x1b[:, c, :],
                                    start=(c == 0), stop=(c == NC - 1))
                        nc.scalar.activation(
                            hT[:, mf:mf + 2, :].rearrange("p a t -> p (a t)"),
                            ps[:], AF.Gelu, bias=zcol[:], scale=1.0)

                # --- FFN2 + residual + LN2 ---
                ps_pools = {0: (ps_main, "mm", 512), 1: (ps_main, "mm", 512),
                            2: (ps_attn, "sc", T), 3: (ps_attn, "sc", T),
                            4: (ps_ctx, "cx", T), 5: (ps_ctx, "cx", T),
                            6: (ps_aux, "ax", 512), 7: (ps_aux, "ax", 512)}
                ps_acc = [ps_pools[m][0].tile([P, ps_pools[m][2]], f32,
                                              name=f"ps_f2_{m}", tag=ps_pools[m][1])
                          for m in range(NC)]
                for fc in range(FC):
                    w2_sb = w2p.tile([P, D], bf16, name="w2_sb", tag="w2")
                    nc.sync.dma_start(w2_sb[:], w2[l, fc * P:(fc + 1) * P, :])
                    for m in range(NC):
                        nc.tensor.matmul(
                            ps_acc[m][:, :T], lhsT=w2_sb[:, m * P:(m + 1) * P],
                            rhs=hT[:, fc, :], start=(fc == 0), stop=(fc == FC - 1))
                ps_mean = ps_aux.tile([P, 512], f32, name="ps_mean", tag="ax")
                ps_sq = ps_aux.tile([P, 512], f32, name="ps_sq", tag="ax")
                for m in range(NC):
                    nc.vector.tensor_add(z[:, m, :], ps_acc[m][:, :T], x1[:, m, :])
                    ln_prep_chunk(m)
                    if m >= 1:
                        ln_stats_chunk(ps_mean, ps_sq, m - 1)
                ln_stats_chunk(ps_mean, ps_sq, NC - 1)
                ln_tail(ps_mean, ps_sq, z, x, xb)

            nc.sync.dma_start(out[:].rearrange("c p t -> p c t"), x[:])

    nc.compile()
    return nc


_NC_CACHE = []


def get_nc():
    if not _NC_CACHE:
        _NC_CACHE.append(build_nc())
    return _NC_CACHE[0]


def prepare_in_maps(inputs):
    inp = {k: np.asarray(v) for k, v in inputs.items()}
    tokens = inp["tokens"]
    emb = inp["emb"].astype(np.float32)

    # host-side embedding lookup + positional encoding (index preprocessing)
    pe = posenc_np(S, D)
    x0 = emb[tokens] + pe[None, :, :]                     # [B, S, D] f32

    # fold attention scale into Wq (scale is a power of two: exact in bf16)
    wq_h = np.ascontiguousarray((inp["Wq"].astype(np.float32) * SCALE)
                                .astype(ml_dtypes.bfloat16))
    wk_h = np.ascontiguousarray(inp["Wk"].astype(np.float32).astype(ml_dtypes.bfloat16))
    wv_h = np.ascontiguousarray(inp["Wv"].astype(np.float32).astype(ml_dtypes.bfloat16))
    wo_h = np.ascontiguousarray(inp["Wo"].astype(np.float32).astype(ml_dtypes.bfloat16))
    w1_h = np.ascontiguousarray(inp["W1"].astype(np.float32).astype(ml_dtypes.bfloat16))
    w2_h = np.ascontiguousarray(inp["W2"].astype(np.float32).astype(ml_dtypes.bfloat16))

    for name in ("bq", "bk", "bv", "bo"):
        assert not np.any(inp[name]), f"nonzero bias {name} not supported"
    assert np.all(inp["ln1_g"] == 1.0) and not np.any(inp["ln1_b"])
    assert np.all(inp["ln2_g"] == 1.0) and not np.any(inp["ln2_b"])

    in_maps = []
    for core in range(NCORES):
        g, r = core // NSH, core % NSH
        xs = x0[g, r * T:(r + 1) * T, :]                  # [T, D]
        x0t = np.ascontiguousarray(
            xs.T.reshape(NC, P, T).astype(np.float32))    # [NC, P, T]
        mb = np.where(tokens[g] == PAD, np.float32(-1e9), np.float32(0.0))
        maskcol = np.ascontiguousarray(mb.reshape(NC, P).T)  # [P, NC]
        in_maps.append({
            "x0t": x0t, "maskcol": maskcol,
            "wq": wq_h, "wk": wk_h, "wv": wv_h, "wo": wo_h,
            "w1": w1_h, "w2": w2_h,
        })
    return in_maps


def assemble_output(res):
    outp = np.empty((B, S, D), np.float32)
    for core in range(NCORES):
        g, r = core // NSH, core % NSH
        o = res.results[core]["out"]                      # [NC, P, T]
        outp[g, r * T:(r + 1) * T, :] = o.reshape(D, T).T
    return outp


def kernel(**inputs):
    nc = get_nc()
    in_maps = prepare_in_maps(inputs)
    res = run_bass_kernel_spmd(nc, in_maps, core_ids=list(range(NCORES)))
    return assemble_output(res)


# revision 51
# speedup vs baseline: 1.0095x; 1.0095x over previous
"""Trainium2 Bass kernel for a 4-layer post-LN transformer encoder.

Sharding: 8 cores = 2 batch groups x 4-way sequence shard (256 tokens/core).
Per layer each core computes QKV for its own tokens, AllGathers K,V within
its 4-core batch group, then runs attention/FFN for its tokens only.
Activations are kept feature-major ([D, T]) on chip so every matmul consumes
natural layouts (weights as lhsT, activations as rhs) with zero transposes.
Matmuls run in bf16 (weights pre-cast on host), accumulation fp32 in PSUM;
softmax and layernorm run in fp32.

Attention per head: scores are computed transposed ([k_tokens, q_tokens]) so
the pad-mask (a per-key bias, i.e. per-partition) and the exp both fold into
the single PSUM-eviction activation. The softmax denominator falls out of the
ctx matmul via a ones-column interleaved into V; division uses a reciprocal
row broadcast across partitions with a tiny K=2 PE outer-product.
"""

import numpy as np
import ml_dtypes

import concourse.bass as bass
import concourse.mybir as mybir
import concourse.tile as tile
from concourse import bacc
from concourse.bass_utils import run_bass_kernel_spmd

# model dims (hardcoded per problem spec)
B, S, D, H, DK, DV, F, L, V = 2, 1024, 1024, 16, 64, 64, 4096, 4, 32000
PAD = 0
EPS = 1e-5
P = 128
NCORES = 8
GROUPS = [[0, 1, 2, 3], [4, 5, 6, 7]]
T = S // 4          # tokens per core
NC = D // P         # feature chunks (8)
FC = F // P         # ffn feature chunks (32)
NSH = 4             # shards per group
TC = T // P         # token chunks per core (2)
SCALE = 1.0 / np.sqrt(DK)

f32 = mybir.dt.float32
bf16 = mybir.dt.bfloat16

AF = mybir.ActivationFunctionType
ALU = mybir.AluOpType


def posenc_np(seq_len, dim):
    pos = np.arange(seq_len, dtype=np.float32)[:, None]
    div = np.exp(-(np.arange(0, dim, 2, dtype=np.float32) / dim) * np.log(10000.0))
    pe = np.zeros((seq_len, dim), np.float32)
    pe[:, 0::2] = np.sin(pos * div)
    pe[:, 1::2] = np.cos(pos * div)
    return pe


def build_nc():
    nc = bacc.Bacc(None, target_bir_lowering=False, num_devices=NCORES)

    # ---- DRAM parameters (per-core) ----
    x0t = nc.declare_dram_parameter("x0t", [NC, P, T], f32, isOutput=False)
    maskcol = nc.declare_dram_parameter("maskcol", [P, NC], f32, isOutput=False)
    wq = nc.declare_dram_parameter("wq", [L, D, D], bf16, isOutput=False)
    wk = nc.declare_dram_parameter("wk", [L, D, D], bf16, isOutput=False)
    wv = nc.declare_dram_parameter("wv", [L, D, D], bf16, isOutput=False)
    wo = nc.declare_dram_parameter("wo", [L, D, D], bf16, isOutput=False)
    w1 = nc.declare_dram_parameter("w1", [L, D, F], bf16, isOutput=False)
    w2 = nc.declare_dram_parameter("w2", [L, F, D], bf16, isOutput=False)
    out = nc.declare_dram_parameter("out", [NC, P, T], f32, isOutput=True)

    with tile.TileContext(nc) as tc:
        with (
            tc.tile_pool(name="persist", bufs=1) as persist,
            tc.tile_pool(name="wp", bufs=12) as wp,
            tc.tile_pool(name="w1p", bufs=10) as w1p,
            tc.tile_pool(name="w2p", bufs=6) as w2p,
            tc.tile_pool(name="pTp", bufs=3) as pTp,
            tc.tile_pool(name="rows", bufs=3) as rows,
            tc.tile_pool(name="ps_main", bufs=2, space="PSUM") as ps_main,
            tc.tile_pool(name="ps_attn", bufs=2, space="PSUM") as ps_attn,
            tc.tile_pool(name="ps_ctx", bufs=2, space="PSUM") as ps_ctx,
            tc.tile_pool(name="ps_aux", bufs=2, space="PSUM") as ps_aux,
            tc.tile_pool(name="dram", bufs=1, space="DRAM") as dram,
        ):
            # ---- persistent SBUF state ----
            x = persist.tile([P, NC, T], f32, name="x")           # residual stream
            xb = persist.tile([P, NC, T], bf16, name="xb")        # bf16 copy
            z = persist.tile([P, NC, T], f32, name="z")           # residual sum
            x1 = persist.tile([P, NC, T], f32, name="x1")         # post-LN1
            x1b = persist.tile([P, NC, T], bf16, name="x1b")
            zb = persist.tile([P, NC, T], bf16, name="zb")        # LN scratch
            sq = persist.tile([P, NC, T], bf16, name="sq")        # LN scratch
            qT = persist.tile([P, NC, T], bf16, name="qT")
            kTl = persist.tile([P, NC, T], bf16, name="kTl")      # local K^T
            vl = persist.tile([P, TC, D], bf16, name="vl")        # local V
            kT = persist.tile([P, NC, NSH, T], bf16, name="kT")   # gathered K^T
            vaug = persist.tile([P, NC, H * (DV + 1)], bf16, name="vaug")
            ctx_un = persist.tile([P, NC, T], bf16, name="ctx_un")
            ctxT = persist.tile([P, NC, T], bf16, name="ctxT")
            hT = persist.tile([P, FC, T], bf16, name="hT")
            mask_sb = persist.tile([P, NC], f32, name="mask_sb")
            ones_col = persist.tile([P, 1], bf16, name="ones_col")
            ones_row = persist.tile([1, P], f32, name="ones_row")
            ones_row_bf = persist.tile([1, P], bf16, name="ones_row_bf")
            eps1 = persist.tile([1, 1], f32, name="eps1")
            zcol = persist.tile([P, 1], f32, name="zcol")

            # DRAM bounce buffers for the per-group K and V AllGathers.
            # K is bounced p-major so both the bounce-in DMA and the gathered
            # load use full-row contiguous lines with no strided rearrange.
            k_in = dram.tile([P, NC, T], bf16, name="k_in")
            v_in = dram.tile([TC, P, D], bf16, name="v_in")
            k_out = dram.tile([NSH, P, NC, T], bf16, name="k_out")
            v_out = dram.tile([NSH, TC, P, D], bf16, name="v_out")

            # ---- prologue ----
            nc.sync.dma_start(x[:], x0t[:].rearrange("c p t -> p c t"))
            nc.sync.dma_start(mask_sb[:], maskcol[:])
            nc.vector.memset(ones_col[:], 1.0)
            nc.vector.memset(ones_row[:], 1.0)
            nc.vector.memset(ones_row_bf[:], 1.0)
            nc.vector.memset(eps1[:], EPS)
            nc.vector.memset(zcol[:], 0.0)
            # ones columns interleaved in vaug (written once; V DMAs skip them)
            nc.vector.memset(
                vaug[:].rearrange("p c (h e) -> p c h e", e=DV + 1)[:, :, :, DV:],
                1.0,
            )
            nc.vector.tensor_copy(xb[:], x[:])

            def ln_prep_chunk(m):
                """bf16 cast + square for chunk m (DVE/ACT, off the PE path)."""
                nc.scalar.copy(zb[:, m, :], z[:, m, :])
                nc.vector.tensor_mul(sq[:, m, :], zb[:, m, :], zb[:, m, :])

            def ln_stats_chunk(ps_mean, ps_sq, m):
                """Stats matmuls for chunk m (emitted lag-1 so PE never waits)."""
                nc.tensor.matmul(ps_mean[0:1, :T], lhsT=ones_col[:],
                                 rhs=zb[:, m, :], start=(m == 0), stop=(m == NC - 1))
                nc.tensor.matmul(ps_sq[0:1, :T], lhsT=ones_col[:],
                                 rhs=sq[:, m, :], start=(m == 0), stop=(m == NC - 1))

            def ln_tail(ps_mean, ps_sq, z_in, x_out, xb_out):
                """LayerNorm tail: row math + partition-broadcast + per-chunk
                normalize (per-chunk so the next GEMM phase starts early).
                rstd = exp(-0.5*ln(var+eps)) keeps ACT in the ln/exp table set."""
                m_row = rows.tile([1, T], f32, name="m_row")
                msq = rows.tile([1, T], f32, name="msq")
                var = rows.tile([1, T], f32, name="var")
                rstd = rows.tile([1, T], f32, name="rstd")
                mrs = rows.tile([1, T], f32, name="mrs")
                nc.vector.tensor_scalar_mul(m_row[:], ps_mean[0:1, :T], 1.0 / D)
                nc.vector.tensor_mul(msq[:], m_row[:], m_row[:])
                nc.vector.scalar_tensor_tensor(
                    var[:], in0=ps_sq[0:1, :T], scalar=1.0 / D, in1=msq[:],
                    op0=ALU.mult, op1=ALU.subtract)
                nc.scalar.activation(var[:], var[:], AF.Ln, bias=eps1[:], scale=1.0)
                nc.scalar.activation(rstd[:], var[:], AF.Exp, bias=zcol[0:1, :],
                                     scale=-0.5)
                nc.vector.tensor_mul(mrs[:], m_row[:], rstd[:])
                ps_r = ps_aux.tile([P, 512], f32, name="ps_r", tag="ax")
                ps_m2 = ps_aux.tile([P, 512], f32, name="ps_m2", tag="ax")
                nc.tensor.matmul(ps_r[:, :T], lhsT=ones_row[:], rhs=rstd[:],
                                 start=True, stop=True)
                nc.tensor.matmul(ps_m2[:, :T], lhsT=ones_row[:], rhs=mrs[:],
                                 start=True, stop=True)
                rb = ps_r[:, None, :T].broadcast_to([P, 2, T])
                mb = ps_m2[:, None, :T].broadcast_to([P, 2, T])
                for c in range(0, NC, 2):
                    xo = x_out[:, c:c + 2, :]
                    nc.vector.tensor_mul(xo, z_in[:, c:c + 2, :], rb)
                    nc.vector.tensor_sub(xo, xo, mb)
                    nc.scalar.copy(xb_out[:, c:c + 2, :], xo)

            # ---- layers ----
            for l in range(L):
                # --- K, V projections first (they feed the collective) ---
                wk_sb = [wp.tile([P, D], bf16, name=f"wk_{l}_{c}", tag="w")
                         for c in range(NC)]
                for c in range(NC):
                    nc.sync.dma_start(wk_sb[c][:], wk[l, c * P:(c + 1) * P, :])
                for m in range(0, NC, 2):
                    ps = ps_main.tile([P, 512], f32, name="ps_k", tag="mm")
                    for half in range(2):
                        for c in range(NC):
                            nc.tensor.matmul(
                                ps[:, half * T:(half + 1) * T],
                                lhsT=wk_sb[c][:, (m + half) * P:(m + half + 1) * P],
                                rhs=xb[:, c, :], start=(c == 0), stop=(c == NC - 1))
                    nc.scalar.copy(
                        kTl[:, m:m + 2, :].rearrange("p a t -> p (a t)"), ps[:])
                nc.sync.dma_start(k_in[:], kTl[:])
                nc.gpsimd.collective_compute(
                    "AllGather", ALU.bypass, replica_groups=GROUPS,
                    ins=[k_in.opt()], outs=[k_out.opt()])

                wv_sb = [wp.tile([P, D], bf16, name=f"wv_{l}_{c}", tag="w")
                         for c in range(NC)]
                for c in range(NC):
                    nc.sync.dma_start(wv_sb[c][:], wv[l, c * P:(c + 1) * P, :])
                for t in range(TC):
                    for nh in range(2):
                        ps = ps_main.tile([P, 512], f32, name="ps_v", tag="mm")
                        for c in range(NC):
                            nc.tensor.matmul(
                                ps[:], lhsT=xb[:, c, t * P:(t + 1) * P],
                                rhs=wv_sb[c][:, nh * 512:(nh + 1) * 512],
                                start=(c == 0), stop=(c == NC - 1))
                        nc.scalar.copy(vl[:, t, nh * 512:(nh + 1) * 512], ps[:])
                    nc.sync.dma_start(v_in[t], vl[:, t, :])
                nc.gpsimd.collective_compute(
                    "AllGather", ALU.bypass, replica_groups=GROUPS,
                    ins=[v_in.opt()], outs=[v_out.opt()])

                # --- Q projection (overlaps with the AllGather) ---
                wq_sb = [wp.tile([P, D], bf16, name=f"wq_{l}_{c}", tag="w")
                         for c in range(NC)]
                for c in range(NC):
                    nc.sync.dma_start(wq_sb[c][:], wq[l, c * P:(c + 1) * P, :])
                for m in range(0, NC, 2):
                    ps = ps_main.tile([P, 512], f32, name="ps_q", tag="mm")
                    for half in range(2):
                        for c in range(NC):
                            nc.tensor.matmul(
                                ps[:, half * T:(half + 1) * T],
                                lhsT=wq_sb[c][:, (m + half) * P:(m + half + 1) * P],
                                rhs=xb[:, c, :], start=(c == 0), stop=(c == NC - 1))
                    nc.scalar.copy(
                        qT[:, m:m + 2, :].rearrange("p a t -> p (a t)"), ps[:])

                # --- pull this group's gathered K/V shards into SBUF ---
                for sh in range(NSH):
                    nc.sync.dma_start(kT[:, :, sh, :], k_out[sh])
                for kc in range(NC):
                    sh, j = kc // 2, kc % 2
                    nc.sync.dma_start(
                        vaug[:, kc, :].rearrange("p (h e) -> p h e", e=DV + 1)[:, :, :DV],
                        v_out[sh, j].rearrange("p (h e) -> p h e", e=DV))

                # --- attention ---
                e = DV + 1
                ps_b = None
                for h in range(H):
                    po = (h % 2) * DV
                    cc = h // 2
                    pT = pTp.tile([P, NC, T], bf16, name="pT")
                    for kcp in range(0, NC, 2):
                        # two k-chunks share one PSUM bank: halves the PE<->ACT
                        # slot-recycle roundtrips in the scores/exp ping-pong
                        ps_s = ps_attn.tile([P, 512], f32, name="ps_s", tag="sc")
                        for half in range(2):
                            kc = kcp + half
                            sh, j = kc // 2, kc % 2
                            nc.tensor.matmul(
                                ps_s[:, half * T:(half + 1) * T],
                                lhsT=kT[po:po + DV, cc, sh, j * P:(j + 1) * P],
                                rhs=qT[po:po + DV, cc, :], start=True, stop=True)
                        for half in range(2):
                            kc = kcp + half
                            nc.scalar.activation(
                                pT[:, kc, :], ps_s[:, half * T:(half + 1) * T],
                                AF.Exp, bias=mask_sb[:, kc:kc + 1], scale=1.0)
                    ps_c = ps_ctx.tile([P, T], f32, name="ps_c", tag="cx")
                    for kc in range(NC):
                        nc.tensor.matmul(
                            ps_c[:e, :], lhsT=vaug[:, kc, h * e:(h + 1) * e],
                            rhs=pT[:, kc, :], start=(kc == 0), stop=(kc == NC - 1))
                    rp = rows.tile([1, T], f32, name="rp")
                    rp_bf = rows.tile([1, T], bf16, name="rp_bf")
                    nc.vector.reciprocal(rp[:], ps_c[DV:e, :])
                    nc.vector.tensor_copy(rp_bf[:], rp[:])
                    nc.vector.tensor_copy(ctx_un[po:po + DV, cc, :], ps_c[:DV, :])
                    if h % 2 == 0:
                        ps_b = ps_aux.tile([P, 512], f32, name="ps_b", tag="ax")
                    nc.tensor.matmul(ps_b[po:po + DV, :T],
                                     lhsT=ones_row_bf[0:1, :DV], rhs=rp_bf[:],
                                     start=True, stop=True)
                    if h % 2 == 1:
                        i = h // 2
                        nc.vector.tensor_mul(ctxT[:, i, :], ctx_un[:, i, :],
                                             ps_b[:, :T])

                # --- Wo + residual + LN1 ---
                wo_sb = [wp.tile([P, D], bf16, name=f"wo_{l}_{c}", tag="w")
                         for c in range(NC)]
                for c in range(NC):
                    nc.sync.dma_start(wo_sb[c][:], wo[l, c * P:(c + 1) * P, :])
                ps_mean = ps_aux.tile([P, 512], f32, name="ps_mean", tag="ax")
                ps_sq = ps_aux.tile([P, 512], f32, name="ps_sq", tag="ax")
                for m in range(0, NC, 2):
                    ps = ps_main.tile([P, 512], f32, name="ps_o", tag="mm")
                    for half in range(2):
                        for c in range(NC):
                            nc.tensor.matmul(
                                ps[:, half * T:(half + 1) * T],
                                lhsT=wo_sb[c][:, (m + half) * P:(m + half + 1) * P],
                                rhs=ctxT[:, c, :], start=(c == 0), stop=(c == NC - 1))
                    nc.vector.tensor_add(
                        z[:, m:m + 2, :].rearrange("p a t -> p (a t)"), ps[:],
                        x[:, m:m + 2, :].rearrange("p a t -> p (a t)"))
                    ln_prep_chunk(m)
                    ln_prep_chunk(m + 1)
                    if m >= 2:
                        ln_stats_chunk(ps_mean, ps_sq, m - 2)
                        ln_stats_chunk(ps_mean, ps_sq, m - 1)
                ln_stats_chunk(ps_mean, ps_sq, NC - 2)
                ln_stats_chunk(ps_mean, ps_sq, NC - 1)
                ln_tail(ps_mean, ps_sq, z, x1, x1b)

                # --- FFN1 (+gelu) ---
                for g in range(4):
                    w1_sb = [w1p.tile([P, D], bf16, name=f"w1_{l}_{g}_{c}", tag="w1")
                             for c in range(NC)]
                    for c in range(NC):
                        nc.sync.dma_start(
                            w1_sb[c][:], w1[l, c * P:(c + 1) * P, g * D:(g + 1) * D])
                    for mf_l in range(0, NC, 2):
                        mf = g * NC + mf_l
                        ps = ps_main.tile([P, 512], f32, name="ps_f1", tag="mm")
                        for half in range(2):
                            for c in range(NC):
                                nc.tensor.matmul(
                                    ps[:, half * T:(half + 1) * T],
                                    lhsT=w1_sb[c][:, (mf_l + half) * P:
                                                  (mf_l + half + 1) * P],
                                    rhs=x1b[:, c, :],
                                    start=(c == 0), stop=(c == NC - 1))
                        nc.scalar.activation(
                            hT[:, mf:mf + 2, :].rearrange("p a t -> p (a t)"),
                            ps[:], AF.Gelu, bias=zcol[:], scale=1.0)

                # --- FFN2 + residual + LN2 ---
                ps_pools = {0: (ps_main, "mm", 512), 1: (ps_main, "mm", 512),
                            2: (ps_attn, "sc", T), 3: (ps_attn, "sc", T),
                            4: (ps_ctx, "cx", T), 5: (ps_ctx, "cx", T),
                            6: (ps_aux, "ax", 512), 7: (ps_aux, "ax", 512)}
                ps_acc = [ps_pools[m][0].tile([P, ps_pools[m][2]], f32,
                                              name=f"ps_f2_{m}", tag=ps_pools[m][1])
                          for m in range(NC)]
                for fc in range(FC):
                    w2_sb = w2p.tile([P, D], bf16, name="w2_sb", tag="w2")
                    nc.sync.dma_start(w2_sb[:], w2[l, fc * P:(fc + 1) * P, :])
                    for m in range(NC):
                        nc.tensor.matmul(
                            ps_acc[m][:, :T], lhsT=w2_sb[:, m * P:(m + 1) * P],
                            rhs=hT[:, fc, :], start=(fc == 0), stop=(fc == FC - 1))
                ps_mean = ps_aux.tile([P, 512], f32, name="ps_mean", tag="ax")
                ps_sq = ps_aux.tile([P, 512], f32, name="ps_sq", tag="ax")
                for m in range(NC):
                    nc.vector.tensor_add(z[:, m, :], ps_acc[m][:, :T], x1[:, m, :])
                    ln_prep_chunk(m)
                    if m >= 1:
                        ln_stats_chunk(ps_mean, ps_sq, m - 1)
                ln_stats_chunk(ps_mean, ps_sq, NC - 1)
                ln_tail(ps_mean, ps_sq, z, x, xb)

            nc.sync.dma_start(out[:].rearrange("c p t -> p c t"), x[:])

    nc.compile()
    return nc


_NC_CACHE = []


def get_nc():
    if not _NC_CACHE:
        _NC_CACHE.append(build_nc())
    return _NC_CACHE[0]


def prepare_in_maps(inputs):
    inp = {k: np.asarray(v) for k, v in inputs.items()}
    tokens = inp["tokens"]
    emb = inp["emb"].astype(np.float32)

    # host-side embedding lookup + positional encoding (index preprocessing)
    pe = posenc_np(S, D)
    x0 = emb[tokens] + pe[None, :, :]                     # [B, S, D] f32

    # fold attention scale into Wq (scale is a power of two: exact in bf16)
    wq_h = np.ascontiguousarray((inp["Wq"].astype(np.float32) * SCALE)
                                .astype(ml_dtypes.bfloat16))
    wk_h = np.ascontiguousarray(inp["Wk"].astype(np.float32).astype(ml_dtypes.bfloat16))
    wv_h = np.ascontiguousarray(inp["Wv"].astype(np.float32).astype(ml_dtypes.bfloat16))
    wo_h = np.ascontiguousarray(inp["Wo"].astype(np.float32).astype(ml_dtypes.bfloat16))
    w1_h = np.ascontiguousarray(inp["W1"].astype(np.float32).astype(ml_dtypes.bfloat16))
    w2_h = np.ascontiguousarray(inp["W2"].astype(np.float32).astype(ml_dtypes.bfloat16))

    for name in ("bq", "bk", "bv", "bo"):
        assert not np.any(inp[name]), f"nonzero bias {name} not supported"
    assert np.all(inp["ln1_g"] == 1.0) and not np.any(inp["ln1_b"])
    assert np.all(inp["ln2_g"] == 1.0) and not np.any(inp["ln2_b"])

    in_maps = []
    for core in range(NCORES):
        g, r = core // NSH, core % NSH
        xs = x0[g, r * T:(r + 1) * T, :]                  # [T, D]
        x0t = np.ascontiguousarray(
            xs.T.reshape(NC, P, T).astype(np.float32))    # [NC, P, T]
        mb = np.where(tokens[g] == PAD, np.float32(-1e9), np.float32(0.0))
        maskcol = np.ascontiguousarray(mb.reshape(NC, P).T)  # [P, NC]
        in_maps.append({
            "x0t": x0t, "maskcol": maskcol,
            "wq": wq_h, "wk": wk_h, "wv": wv_h, "wo": wo_h,
            "w1": w1_h, "w2": w2_h,
        })
    return in_maps


def assemble_output(res):
    outp = np.empty((B, S, D), np.float32)
    for core in range(NCORES):
        g, r = core // NSH, core % NSH
        o = res.results[core]["out"]                      # [NC, P, T]
        outp[g, r * T:(r + 1) * T, :] = o.reshape(D, T).T
    return outp


def kernel(**inputs):
    nc = get_nc()
    in_maps = prepare_in_maps(inputs)
    res = run_bass_kernel_spmd(nc, in_maps, core_ids=list(range(NCORES)))
    return assemble_output(res)


# revision 53
# speedup vs baseline: 1.0345x; 1.0248x over previous
"""Trainium2 Bass kernel for a 4-layer post-LN transformer encoder.

Sharding: 8 cores = 2 batch groups x 4-way sequence shard (256 tokens/core).
Per layer each core computes QKV for its own tokens, AllGathers K,V within
its 4-core batch group, then runs attention/FFN for its tokens only.
Activations are kept feature-major ([D, T]) on chip so every matmul consumes
natural layouts (weights as lhsT, activations as rhs) with zero transposes.
Matmuls run in bf16 (weights pre-cast on host), accumulation fp32 in PSUM;
softmax and layernorm run in fp32.

Attention per head: scores are computed transposed ([k_tokens, q_tokens]) so
the pad-mask (a per-key bias, i.e. per-partition) and the exp both fold into
the single PSUM-eviction activation. The softmax denominator falls out of the
ctx matmul via a ones-column interleaved into V; division uses a reciprocal
row broadcast across partitions with a tiny K=2 PE outer-product.
"""

import numpy as np
import ml_dtypes

import concourse.bass as bass
import concourse.mybir as mybir
import concourse.tile as tile
from concourse import bacc
from concourse.bass_utils import run_bass_kernel_spmd

# model dims (hardcoded per problem spec)
B, S, D, H, DK, DV, F, L, V = 2, 1024, 1024, 16, 64, 64, 4096, 4, 32000
PAD = 0
EPS = 1e-5
P = 128
NCORES = 8
GROUPS = [[0, 1, 2, 3], [4, 5, 6, 7]]
T = S // 4          # tokens per core
NC = D // P         # feature chunks (8)
FC = F // P         # ffn feature chunks (32)
NSH = 4             # shards per group
TC = T // P         # token chunks per core (2)
SCALE = 1.0 / np.sqrt(DK)

f32 = mybir.dt.float32
bf16 = mybir.dt.bfloat16

AF = mybir.ActivationFunctionType
ALU = mybir.AluOpType


def posenc_np(seq_len, dim):
    pos = np.arange(seq_len, dtype=np.float32)[:, None]
    div = np.exp(-(np.arange(0, dim, 2, dtype=np.float32) / dim) * np.log(10000.0))
    pe = np.zeros((seq_len, dim), np.float32)
    pe[:, 0::2] = np.sin(pos * div)
    pe[:, 1::2] = np.cos(pos * div)
    return pe


def build_nc():
    nc = bacc.Bacc(None, target_bir_lowering=False, num_devices=NCORES)

    # ---- DRAM parameters (per-core) ----
    x0t = nc.declare_dram_parameter("x0t", [NC, P, T], f32, isOutput=False)
    maskcol = nc.declare_dram_parameter("maskcol", [P, NC], f32, isOutput=False)
    wq = nc.declare_dram_parameter("wq", [L, D, D], bf16, isOutput=False)
    wk = nc.declare_dram_parameter("wk", [L, D, D], bf16, isOutput=False)
    wv = nc.declare_dram_parameter("wv", [L, D, D], bf16, isOutput=False)
    wo = nc.declare_dram_parameter("wo", [L, D, D], bf16, isOutput=False)
    w1 = nc.declare_dram_parameter("w1", [L, D, F], bf16, isOutput=False)
    w2 = nc.declare_dram_parameter("w2", [L, F, D], bf16, isOutput=False)
    out = nc.declare_dram_parameter("out", [NC, P, T], f32, isOutput=True)

    with tile.TileContext(nc) as tc:
        with (
            tc.tile_pool(name="persist", bufs=1) as persist,
            tc.tile_pool(name="wp", bufs=12) as wp,
            tc.tile_pool(name="w1p", bufs=10) as w1p,
            tc.tile_pool(name="w2p", bufs=6) as w2p,
            tc.tile_pool(name="pTp", bufs=4) as pTp,
            tc.tile_pool(name="rows", bufs=3) as rows,
            tc.tile_pool(name="ps_main", bufs=2, space="PSUM") as ps_main,
            tc.tile_pool(name="ps_attn", bufs=2, space="PSUM") as ps_attn,
            tc.tile_pool(name="ps_ctx", bufs=2, space="PSUM") as ps_ctx,
            tc.tile_pool(name="ps_aux", bufs=2, space="PSUM") as ps_aux,
            tc.tile_pool(name="dram", bufs=1, space="DRAM") as dram,
        ):
            # ---- persistent SBUF state ----
            x = persist.tile([P, NC, T], f32, name="x")           # residual stream
            xb = persist.tile([P, NC, T], bf16, name="xb")        # bf16 copy
            z = persist.tile([P, NC, T], f32, name="z")           # residual sum
            x1 = persist.tile([P, NC, T], f32, name="x1")         # post-LN1
            x1b = persist.tile([P, NC, T], bf16, name="x1b")
            zb = persist.tile([P, NC, T], bf16, name="zb")        # LN scratch
            sq = persist.tile([P, NC, T], bf16, name="sq")        # LN scratch
            qT = persist.tile([P, NC, T], bf16, name="qT")
            kTl = persist.tile([P, NC, T], bf16, name="kTl")      # local K^T
            vl = persist.tile([P, TC, D], bf16, name="vl")        # local V
            kT = persist.tile([P, NC, NSH, T], bf16, name="kT")   # gathered K^T
            vaug = persist.tile([P, NC, H * (DV + 1)], bf16, name="vaug")
            ctx_un = persist.tile([P, NC, T], bf16, name="ctx_un")
            ctxT = persist.tile([P, NC, T], bf16, name="ctxT")
            hT = persist.tile([P, FC, T], bf16, name="hT")
            mask_sb = persist.tile([P, NC], f32, name="mask_sb")
            ones_col = persist.tile([P, 1], bf16, name="ones_col")
            ones_row = persist.tile([1, P], f32, name="ones_row")
            ones_row_bf = persist.tile([1, P], bf16, name="ones_row_bf")
            eps1 = persist.tile([1, 1], f32, name="eps1")
            zcol = persist.tile([P, 1], f32, name="zcol")

            # DRAM bounce buffers for the per-group K and V AllGathers.
            # K is bounced p-major so both the bounce-in DMA and the gathered
            # load use full-row contiguous lines with no strided rearrange.
            k_in = dram.tile([P, NC, T], bf16, name="k_in")
            v_in = dram.tile([TC, P, D], bf16, name="v_in")
            k_out = dram.tile([NSH, P, NC, T], bf16, name="k_out")
            v_out = dram.tile([NSH, TC, P, D], bf16, name="v_out")

            # ---- prologue ----
            nc.sync.dma_start(x[:], x0t[:].rearrange("c p t -> p c t"))
            nc.sync.dma_start(mask_sb[:], maskcol[:])
            nc.vector.memset(ones_col[:], 1.0)
            nc.vector.memset(ones_row[:], 1.0)
            nc.vector.memset(ones_row_bf[:], 1.0)
            nc.vector.memset(eps1[:], EPS)
            nc.vector.memset(zcol[:], 0.0)
            # ones columns interleaved in vaug (written once; V DMAs skip them)
            nc.vector.memset(
                vaug[:].rearrange("p c (h e) -> p c h e", e=DV + 1)[:, :, :, DV:],
                1.0,
            )
            nc.vector.tensor_copy(xb[:], x[:])

            def ln_prep_chunk(m):
                """bf16 cast + square for chunk m (DVE/ACT, off the PE path)."""
                nc.scalar.copy(zb[:, m, :], z[:, m, :])
                nc.vector.tensor_mul(sq[:, m, :], zb[:, m, :], zb[:, m, :])

            def ln_stats_chunk(ps_mean, ps_sq, m):
                """Stats matmuls for chunk m (emitted lag-1 so PE never waits)."""
                nc.tensor.matmul(ps_mean[0:1, :T], lhsT=ones_col[:],
                                 rhs=zb[:, m, :], start=(m == 0), stop=(m == NC - 1))
                nc.tensor.matmul(ps_sq[0:1, :T], lhsT=ones_col[:],
                                 rhs=sq[:, m, :], start=(m == 0), stop=(m == NC - 1))

            def ln_tail(ps_mean, ps_sq, z_in, x_out, xb_out):
                """LayerNorm tail: row math + partition-broadcast + per-chunk
                normalize (per-chunk so the next GEMM phase starts early).
                rstd = exp(-0.5*ln(var+eps)) keeps ACT in the ln/exp table set."""
                m_row = rows.tile([1, T], f32, name="m_row")
                msq = rows.tile([1, T], f32, name="msq")
                var = rows.tile([1, T], f32, name="var")
                rstd = rows.tile([1, T], f32, name="rstd")
                mrs = rows.tile([1, T], f32, name="mrs")
                nc.vector.tensor_scalar_mul(m_row[:], ps_mean[0:1, :T], 1.0 / D)
                nc.vector.tensor_mul(msq[:], m_row[:], m_row[:])
                nc.vector.scalar_tensor_tensor(
                    var[:], in0=ps_sq[0:1, :T], scalar=1.0 / D, in1=msq[:],
                    op0=ALU.mult, op1=ALU.subtract)
                nc.scalar.activation(var[:], var[:], AF.Ln, bias=eps1[:], scale=1.0)
                nc.scalar.activation(rstd[:], var[:], AF.Exp, bias=zcol[0:1, :],
                                     scale=-0.5)
                nc.vector.tensor_mul(mrs[:], m_row[:], rstd[:])
                ps_r = ps_aux.tile([P, 512], f32, name="ps_r", tag="ax")
                ps_m2 = ps_aux.tile([P, 512], f32, name="ps_m2", tag="ax")
                nc.tensor.matmul(ps_r[:, :T], lhsT=ones_row[:], rhs=rstd[:],
                                 start=True, stop=True)
                nc.tensor.matmul(ps_m2[:, :T], lhsT=ones_row[:], rhs=mrs[:],
                                 start=True, stop=True)
                rb = ps_r[:, None, :T].broadcast_to([P, 2, T])
                mb = ps_m2[:, None, :T].broadcast_to([P, 2, T])
                for c in range(0, NC, 2):
                    xo = x_out[:, c:c + 2, :]
                    nc.vector.tensor_mul(xo, z_in[:, c:c + 2, :], rb)
                    nc.vector.tensor_sub(xo, xo, mb)
                    nc.scalar.copy(xb_out[:, c:c + 2, :], xo)

            # ---- layers ----
            for l in range(L):
                # --- K, V projections first (they feed the collective) ---
                wk_sb = [wp.tile([P, D], bf16, name=f"wk_{l}_{c}", tag="w")
                         for c in range(NC)]
                for c in range(NC):
                    nc.sync.dma_start(wk_sb[c][:], wk[l, c * P:(c + 1) * P, :])
                for m in range(0, NC, 2):
                    ps = ps_main.tile([P, 512], f32, name="ps_k", tag="mm")
                    for half in range(2):
                        for c in range(NC):
                            nc.tensor.matmul(
                                ps[:, half * T:(half + 1) * T],
                                lhsT=wk_sb[c][:, (m + half) * P:(m + half + 1) * P],
                                rhs=xb[:, c, :], start=(c == 0), stop=(c == NC - 1))
                    nc.scalar.copy(
                        kTl[:, m:m + 2, :].rearrange("p a t -> p (a t)"), ps[:])
                nc.sync.dma_start(k_in[:], kTl[:])
                nc.gpsimd.collective_compute(
                    "AllGather", ALU.bypass, replica_groups=GROUPS,
                    ins=[k_in.opt()], outs=[k_out.opt()])

                wv_sb = [wp.tile([P, D], bf16, name=f"wv_{l}_{c}", tag="w")
                         for c in range(NC)]
                for c in range(NC):
                    nc.sync.dma_start(wv_sb[c][:], wv[l, c * P:(c + 1) * P, :])
                for t in range(TC):
                    for nh in range(2):
                        ps = ps_main.tile([P, 512], f32, name="ps_v", tag="mm")
                        for c in range(NC):
                            nc.tensor.matmul(
                                ps[:], lhsT=xb[:, c, t * P:(t + 1) * P],
                                rhs=wv_sb[c][:, nh * 512:(nh + 1) * 512],
                                start=(c == 0), stop=(c == NC - 1))
                        nc.scalar.copy(vl[:, t, nh * 512:(nh + 1) * 512], ps[:])
                    nc.sync.dma_start(v_in[t], vl[:, t, :])
                nc.gpsimd.collective_compute(
                    "AllGather", ALU.bypass, replica_groups=GROUPS,
                    ins=[v_in.opt()], outs=[v_out.opt()])

                # --- Q projection (overlaps with the AllGather) ---
                wq_sb = [wp.tile([P, D], bf16, name=f"wq_{l}_{c}", tag="w")
                         for c in range(NC)]
                for c in range(NC):
                    nc.sync.dma_start(wq_sb[c][:], wq[l, c * P:(c + 1) * P, :])
                for m in range(0, NC, 2):
                    ps = ps_main.tile([P, 512], f32, name="ps_q", tag="mm")
                    for half in range(2):
                        for c in range(NC):
                            nc.tensor.matmul(
                                ps[:, half * T:(half + 1) * T],
                                lhsT=wq_sb[c][:, (m + half) * P:(m + half + 1) * P],
                                rhs=xb[:, c, :], start=(c == 0), stop=(c == NC - 1))
                    nc.scalar.copy(
                        qT[:, m:m + 2, :].rearrange("p a t -> p (a t)"), ps[:])

                # --- pull this group's gathered K/V shards into SBUF ---
                for sh in range(NSH):
                    nc.sync.dma_start(kT[:, :, sh, :], k_out[sh])
                for kc in range(NC):
                    sh, j = kc // 2, kc % 2
                    nc.sync.dma_start(
                        vaug[:, kc, :].rearrange("p (h e) -> p h e", e=DV + 1)[:, :, :DV],
                        v_out[sh, j].rearrange("p (h e) -> p h e", e=DV))

                # --- attention ---
                e = DV + 1
                ps_b = None
                for h in range(H):
                    po = (h % 2) * DV
                    cc = h // 2
                    pT = pTp.tile([P, NC, T], bf16, name="pT")
                    for kcp in range(0, NC, 2):
                        # two k-chunks share one PSUM bank: halves the PE<->ACT
                        # slot-recycle roundtrips in the scores/exp ping-pong
                        ps_s = ps_attn.tile([P, 512], f32, name="ps_s", tag="sc")
                        for half in range(2):
                            kc = kcp + half
                            sh, j = kc // 2, kc % 2
                            nc.tensor.matmul(
                                ps_s[:, half * T:(half + 1) * T],
                                lhsT=kT[po:po + DV, cc, sh, j * P:(j + 1) * P],
                                rhs=qT[po:po + DV, cc, :], start=True, stop=True)
                        for half in range(2):
                            kc = kcp + half
                            nc.scalar.activation(
                                pT[:, kc, :], ps_s[:, half * T:(half + 1) * T],
                                AF.Exp, bias=mask_sb[:, kc:kc + 1], scale=1.0)
                    # alternate ctx accumulators between the cx pool and the
                    # (idle during attention) mm pool: 4-deep head pipeline
                    if h % 2 == 0:
                        ps_c = ps_ctx.tile([P, T], f32, name="ps_c", tag="cx")
                    else:
                        ps_c = ps_main.tile([P, T], f32, name="ps_c2", tag="mm")
                    for kc in range(NC):
                        nc.tensor.matmul(
                            ps_c[:e, :], lhsT=vaug[:, kc, h * e:(h + 1) * e],
                            rhs=pT[:, kc, :], start=(kc == 0), stop=(kc == NC - 1))
                    rp = rows.tile([1, T], f32, name="rp")
                    rp_bf = rows.tile([1, T], bf16, name="rp_bf")
                    nc.vector.reciprocal(rp[:], ps_c[DV:e, :])
                    nc.vector.tensor_copy(rp_bf[:], rp[:])
                    nc.vector.tensor_copy(ctx_un[po:po + DV, cc, :], ps_c[:DV, :])
                    if h % 2 == 0:
                        ps_b = ps_aux.tile([P, 512], f32, name="ps_b", tag="ax")
                    nc.tensor.matmul(ps_b[po:po + DV, :T],
                                     lhsT=ones_row_bf[0:1, :DV], rhs=rp_bf[:],
                                     start=True, stop=True)
                    if h % 2 == 1:
                        i = h // 2
                        nc.vector.tensor_mul(ctxT[:, i, :], ctx_un[:, i, :],
                                             ps_b[:, :T])

                # --- Wo + residual + LN1 ---
                wo_sb = [wp.tile([P, D], bf16, name=f"wo_{l}_{c}", tag="w")
                         for c in range(NC)]
                for c in range(NC):
                    nc.sync.dma_start(wo_sb[c][:], wo[l, c * P:(c + 1) * P, :])
                ps_mean = ps_aux.tile([P, 512], f32, name="ps_mean", tag="ax")
                ps_sq = ps_aux.tile([P, 512], f32, name="ps_sq", tag="ax")
                for m in range(0, NC, 2):
                    ps = ps_main.tile([P, 512], f32, name="ps_o", tag="mm")
                    for half in range(2):
                        for c in range(NC):
                            nc.tensor.matmul(
                                ps[:, half * T:(half + 1) * T],
                                lhsT=wo_sb[c][:, (m + half) * P:(m + half + 1) * P],
                                rhs=ctxT[:, c, :], start=(c == 0), stop=(c == NC - 1))
                    nc.vector.tensor_add(
                        z[:, m:m + 2, :].rearrange("p a t -> p (a t)"), ps[:],
                        x[:, m:m + 2, :].rearrange("p a t -> p (a t)"))
                    ln_prep_chunk(m)
                    ln_prep_chunk(m + 1)
                    if m >= 2:
                        ln_stats_chunk(ps_mean, ps_sq, m - 2)
                        ln_stats_chunk(ps_mean, ps_sq, m - 1)
                ln_stats_chunk(ps_mean, ps_sq, NC - 2)
                ln_stats_chunk(ps_mean, ps_sq, NC - 1)
                ln_tail(ps_mean, ps_sq, z, x1, x1b)

                # --- FFN1 (+gelu) ---
                for g in range(4):
                    w1_sb = [w1p.tile([P, D], bf16, name=f"w1_{l}_{g}_{c}", tag="w1")
                             for c in range(NC)]
                    for c in range(NC):
                        nc.sync.dma_start(
                            w1_sb[c][:], w1[l, c * P:(c + 1) * P, g * D:(g + 1) * D])
                    for mf_l in range(0, NC, 2):
                        mf = g * NC + mf_l
                        ps = ps_main.tile([P, 512], f32, name="ps_f1", tag="mm")
                        for half in range(2):
                            for c in range(NC):
                                nc.tensor.matmul(
                                    ps[:, half * T:(half + 1) * T],
                                    lhsT=w1_sb[c][:, (mf_l + half) * P:
                                                  (mf_l + half + 1) * P],
                                    rhs=x1b[:, c, :],
                                    start=(c == 0), stop=(c == NC - 1))
                        nc.scalar.activation(
                            hT[:, mf:mf + 2, :].rearrange("p a t -> p (a t)"),
                            ps[:], AF.Gelu, bias=zcol[:], scale=1.0)

                # --- FFN2 + residual + LN2 ---
                ps_pools = {0: (ps_main, "mm", 512), 1: (ps_main, "mm", 512),
                            2: (ps_attn, "sc", T), 3: (ps_attn, "sc", T),
                            4: (ps_ctx, "cx", T), 5: (ps_ctx, "cx", T),
                            6: (ps_aux, "ax", 512), 7: (ps_aux, "ax", 512)}
                ps_acc = [ps_pools[m][0].tile([P, ps_pools[m][2]], f32,
                                              name=f"ps_f2_{m}", tag=ps_pools[m][1])
                          for m in range(NC)]
                for fc in range(FC):
                    w2_sb = w2p.tile([P, D], bf16, name="w2_sb", tag="w2")
                    nc.sync.dma_start(w2_sb[:], w2[l, fc * P:(fc + 1) * P, :])
                    for m in range(NC):
                        nc.tensor.matmul(
                            ps_acc[m][:, :T], lhsT=w2_sb[:, m * P:(m + 1) * P],
                            rhs=hT[:, fc, :], start=(fc == 0), stop=(fc == FC - 1))
                ps_mean = ps_aux.tile([P, 512], f32, name="ps_mean", tag="ax")
                ps_sq = ps_aux.tile([P, 512], f32, name="ps_sq", tag="ax")
                for m in range(NC):
                    nc.vector.tensor_add(z[:, m, :], ps_acc[m][:, :T], x1[:, m, :])
                    ln_prep_chunk(m)
                    if m >= 1:
                        ln_stats_chunk(ps_mean, ps_sq, m - 1)
                ln_stats_chunk(ps_mean, ps_sq, NC - 1)
                ln_tail(ps_mean, ps_sq, z, x, xb)

            nc.sync.dma_start(out[:].rearrange("c p t -> p c t"), x[:])

    nc.compile()
    return nc


_NC_CACHE = []


def get_nc():
    if not _NC_CACHE:
        _NC_CACHE.append(build_nc())
    return _NC_CACHE[0]


def prepare_in_maps(inputs):
    inp = {k: np.asarray(v) for k, v in inputs.items()}
    tokens = inp["tokens"]
    emb = inp["emb"].astype(np.float32)

    # host-side embedding lookup + positional encoding (index preprocessing)
    pe = posenc_np(S, D)
    x0 = emb[tokens] + pe[None, :, :]                     # [B, S, D] f32

    # fold attention scale into Wq (scale is a power of two: exact in bf16)
    wq_h = np.ascontiguousarray((inp["Wq"].astype(np.float32) * SCALE)
                                .astype(ml_dtypes.bfloat16))
    wk_h = np.ascontiguousarray(inp["Wk"].astype(np.float32).astype(ml_dtypes.bfloat16))
    wv_h = np.ascontiguousarray(inp["Wv"].astype(np.float32).astype(ml_dtypes.bfloat16))
    wo_h = np.ascontiguousarray(inp["Wo"].astype(np.float32).astype(ml_dtypes.bfloat16))
    w1_h = np.ascontiguousarray(inp["W1"].astype(np.float32).astype(ml_dtypes.bfloat16))
    w2_h = np.ascontiguousarray(inp["W2"].astype(np.float32).astype(ml_dtypes.bfloat16))

    for name in ("bq", "bk", "bv", "bo"):
        assert not np.any(inp[name]), f"nonzero bias {name} not supported"
    assert np.all(inp["ln1_g"] == 1.0) and not np.any(inp["ln1_b"])
    assert np.all(inp["ln2_g"] == 1.0) and not np.any(inp["ln2_b"])

    in_maps = []
    for core in range(NCORES):
        g, r = core // NSH, core % NSH
        xs = x0[g, r * T:(r + 1) * T, :]                  # [T, D]
        x0t = np.ascontiguousarray(
            xs.T.reshape(NC, P, T).astype(np.float32))    # [NC, P, T]
        mb = np.where(tokens[g] == PAD, np.float32(-1e9), np.float32(0.0))
        maskcol = np.ascontiguousarray(mb.reshape(NC, P).T)  # [P, NC]
        in_maps.append({
            "x0t": x0t, "maskcol": maskcol,
            "wq": wq_h, "wk": wk_h, "wv": wv_h, "wo": wo_h,
            "w1": w1_h, "w2": w2_h,
        })
    return in_maps


def assemble_output(res):
    outp = np.empty((B, S, D), np.float32)
    for core in range(NCORES):
        g, r = core // NSH, core % NSH
        o = res.results[core]["out"]                      # [NC, P, T]
        outp[g, r * T:(r + 1) * T, :] = o.reshape(D, T).T
    return outp


def kernel(**inputs):
    nc = get_nc()
    in_maps = prepare_in_maps(inputs)
    res = run_bass_kernel_spmd(nc, in_maps, core_ids=list(range(NCORES)))
    return assemble_output(res)
